# revision 37
# baseline (speedup 1.0000x reference)
"""HGT (heterogeneous graph transformer) layer on 8 Trainium2 NeuronCores.

Sharding: data-parallel over destination-paper partitions (8 shards of 25000).
Edge tiles are aligned to static 128-dst windows (S sub-tiles of <=128 edges
per window, host-verified), so every DMA offset is static: q rows live
SBUF-resident and are distributed to edges with a one-hot matmul, per-window
aggregates accumulate in PSUM and are written to dst-indexed DRAM rows that
the final stage reads back sequentially. No indirect/gather DMA anywhere.
"""
import numpy as np
import ml_dtypes

import concourse.bass as bass
import concourse.bacc as bacc
import concourse.mybir as mybir
import concourse.tile as tile
from concourse.bass_utils import run_bass_kernel_spmd
from concourse.masks import make_identity

BF = ml_dtypes.bfloat16
F32 = mybir.dt.float32
BF16 = mybir.dt.bfloat16
AF = mybir.ActivationFunctionType
ALU = mybir.AluOpType

NPP, NA = 200000, 100000
HID, H, D = 128, 4, 32
FP, FA, OUT = 256, 128, 16
SCALE = float(1.0 / np.sqrt(D))

NCORE = 8
SH = 25000
NWIN = 196          # 196*128 = 25088 >= 25000 dst slots per core
PAD_DL = 200.0      # dst-local sentinel for padding edge slots

_BUILD_CACHE = {}


# ---------------------------------------------------------------- host side

def _fold_weights(inp):
    """Fold all weight/bias tensors into the device-facing layout (numpy)."""
    def bd(rel):  # [H,D,D] -> block-diagonal [HID,HID]
        m = np.zeros((HID, HID), np.float32)
        for h in range(H):
            m[h * D:(h + 1) * D, h * D:(h + 1) * D] = rel[h]
        return m

    f = {}
    w1p = np.asarray(inp["lin_paper_w"], np.float32)      # [FP, HID]
    w1a = np.asarray(inp["lin_author_w"], np.float32)     # [FA, HID]
    f["b1p"] = np.asarray(inp["lin_paper_b"], np.float32).reshape(HID, 1)
    f["b1a"] = np.asarray(inp["lin_author_b"], np.float32).reshape(HID, 1)

    ncp = FP // 128
    f["w1p"] = np.ascontiguousarray(
        w1p.reshape(ncp, 128, HID).transpose(1, 0, 2)).astype(BF)
    f["w1a"] = w1a.astype(BF)                              # [128, HID]

    for rel, t, e in (("c", "paper", "cites"), ("w", "author", "writes")):
        bda = bd(np.asarray(inp[f"a_rel_{e}"], np.float32))
        bdm = bd(np.asarray(inp[f"m_rel_{e}"], np.float32))
        scl = np.repeat(np.asarray(inp[f"p_rel_{e}"], np.float32) * SCALE, D)
        wk = (np.asarray(inp[f"k_w_{t}"], np.float32) @ bda) * scl[None, :]
        bk = (np.asarray(inp[f"k_b_{t}"], np.float32) @ bda) * scl
        wv = np.asarray(inp[f"v_w_{t}"], np.float32) @ bdm
        bv = np.asarray(inp[f"v_b_{t}"], np.float32) @ bdm
        f[f"wkv_{rel}"] = np.concatenate([wk, wv], axis=1).astype(BF)  # [HID,256]
        f[f"bkv_{rel}"] = np.concatenate([bk, bv]).reshape(1, 256).astype(BF)

    f["wq"] = np.asarray(inp["q_w_paper"], np.float32).astype(BF)      # [HID,HID]
    f["bq"] = np.asarray(inp["q_b_paper"], np.float32).reshape(1, HID).astype(BF)

    beta = 1.0 / (1.0 + np.exp(-float(np.asarray(inp["skip_paper"]))))
    aw = np.asarray(inp["a_w_paper"], np.float32)
    ab = np.asarray(inp["a_b_paper"], np.float32)
    lo_w = np.asarray(inp["lin_out_w"], np.float32)
    lo_b = np.asarray(inp["lin_out_b"], np.float32)
    wg = beta * (aw @ lo_w)                 # [HID, OUT]
    wh = (1.0 - beta) * lo_w                # [HID, OUT]
    f["wgh"] = np.concatenate([wg, wh], axis=1).astype(BF)  # [HID, 2*OUT]
    f["bfin"] = (beta * (ab @ lo_w) + lo_b).reshape(1, OUT).astype(BF)

    # iota consts for one-hot builds: iorow[p, s*128+e] = e ; iocol[p, 0] = p
    f["iorow"] = np.tile(np.arange(128, dtype=np.float32), (128, 4)).reshape(
        128, 512).astype(BF)
    f["iocol"] = np.arange(128, dtype=np.float32).reshape(128, 1)
    f["flags"] = (
        bool(np.any(f["bkv_c"].astype(np.float32))),
        bool(np.any(f["bkv_w"].astype(np.float32))),
        bool(np.any(f["bq"].astype(np.float32))),
        bool(np.any(f["bfin"].astype(np.float32))),
    )
    return f


def _edge_layout(src, dst, lo, hi, s_max):
    """Window-aligned edge packing for one (core, relation).

    Returns (pad_src[nt_e*128] int64 into x_aug rows, dl[nt_e*128] f32
    window-local dst or PAD_DL) with nt_e = NWIN*s_max; sub-tile t of window
    w occupies slots [(w*s_max+t)*128, ...+128).
    """
    sel = (dst >= lo) & (dst < hi)
    s = src[sel].astype(np.int64)
    d = (dst[sel] - lo).astype(np.int64)
    order = np.argsort(d, kind="stable")
    s, d = s[order], d[order]
    win = d >> 7
    cnt = np.bincount(win, minlength=NWIN)
    assert cnt.max() <= 128 * s_max, (cnt.max(), s_max)
    start = np.zeros(NWIN + 1, np.int64)
    np.cumsum(cnt, out=start[1:])
    nt_e = NWIN * s_max
    pad_src = np.full(nt_e * 128, -1, np.int64)   # -1 -> zero row (filled later)
    dl = np.full(nt_e * 128, PAD_DL, np.float32)
    # slot index for edge i: window w, rank r = i - start[w]
    w_of = win
    r = np.arange(len(d)) - start[w_of]
    slot = (w_of * s_max) * 128 + r
    pad_src[slot] = s
    dl[slot] = d - (w_of << 7)
    return pad_src, dl


def _prep_core(m, edges, xp_aug, xa_aug, s_max):
    """Per-core host arrays. xp_aug/xa_aug: bf16 x tables with zero row last."""
    lo, hi = m * SH, (m + 1) * SH
    nt_e = NWIN * s_max
    out = {}

    def prep_rel(src, dst, x_aug, nchunk, tag):
        pad_src, dl = _edge_layout(src, dst, lo, hi, s_max)
        pad_src[pad_src < 0] = x_aug.shape[0] - 1
        xg = x_aug[pad_src]                                   # [nt_e*128, F]
        out[f"xT_{tag}"] = np.ascontiguousarray(
            xg.reshape(nt_e, 128, nchunk, 128).transpose(0, 3, 2, 1))
        # dl per sub-tile on partitions: [nt_e, 128] -> [nb, 128, 4]
        dlt = dl.reshape(nt_e // 4, 4, 128)
        out[f"dl_{tag}"] = np.ascontiguousarray(dlt.transpose(0, 2, 1)).astype(BF)
        out[f"dlr_{tag}"] = np.ascontiguousarray(
            dl.reshape(nt_e // 4, 1, 512)).astype(BF)

    ws, wd = edges["ws"], edges["wd"]
    cs, cd = edges["cs"], edges["cd"]
    prep_rel(cs, cd, xp_aug, FP // 128, "c")
    prep_rel(ws, wd, xa_aug, FA // 128, "w")

    own_idx = np.full(NWIN * 128, xp_aug.shape[0] - 1, np.int64)
    own_idx[:SH] = np.arange(lo, hi)
    xg = xp_aug[own_idx]
    out["xT_own"] = np.ascontiguousarray(
        xg.reshape(NWIN, 128, FP // 128, 128).transpose(0, 3, 2, 1))
    return out


# -------------------------------------------------------------- device side

def build_program(s_max, flags, debug=False):
    """Build + compile the SPMD program (structure shared by all cores)."""
    nt_e = NWIN * s_max
    nb_e = nt_e // 4            # half-batches of 4 sub-tiles = 2 windows
    ncp, nca = FP // 128, FA // 128
    f_bkv_c, f_bkv_w, f_bq, f_bfin = flags

    nc = bacc.Bacc()
    P = {}
    def par(name, shape, dt):
        P[name] = nc.declare_dram_parameter(name, list(shape), dt, isOutput=False)
        return P[name]

    par("xT_own", [NWIN, 128, ncp, 128], BF16)
    par("xT_c", [nt_e, 128, ncp, 128], BF16)
    par("xT_w", [nt_e, 128, nca, 128], BF16)
    par("dl_c", [nb_e, 128, 4], BF16)
    par("dl_w", [nb_e, 128, 4], BF16)
    par("dlr_c", [nb_e, 1, 512], BF16)
    par("dlr_w", [nb_e, 1, 512], BF16)
    par("w1p", [128, ncp, 128], BF16)
    par("w1a", [128, nca, 128], BF16)
    par("b1p", [128, 1], F32)
    par("b1a", [128, 1], F32)
    par("wkv_c", [128, 256], BF16)
    par("wkv_w", [128, 256], BF16)
    par("bkv_c", [1, 256], BF16)
    par("bkv_w", [1, 256], BF16)
    par("wq", [128, 128], BF16)
    par("bq", [1, 128], BF16)
    par("wgh", [128, 2 * OUT], BF16)
    par("bfin", [1, OUT], BF16)
    par("iorow", [128, 512], BF16)
    par("iocol", [128, 1], F32)
    out_t = nc.declare_dram_parameter("out", [NWIN * 128, OUT], F32, isOutput=True)

    # per-relation dst-indexed tables: [slot, 0:128]=sum(v*e), [128:132]=sum(e)
    ctab_c = nc.dram_tensor("ctab_c", [NWIN, 128, 136], BF16)
    ctab_w = nc.dram_tensor("ctab_w", [NWIN, 128, 136], BF16)
    if debug:
        dbg_cc = nc.declare_dram_parameter("dbg_cc", [NWIN * 128, 136], BF16, isOutput=True)
        dbg_q = nc.declare_dram_parameter("dbg_q", [NWIN * 128, 128], BF16, isOutput=True)

    with tile.TileContext(nc) as tc:
        with tc.tile_pool(name="const", bufs=1) as cp:
            def cload(name, shape2d, dt, ap=None):
                t = cp.tile(list(shape2d), dt, tag=f"const_{name}")
                nc.sync.dma_start(out=t[:], in_=(ap if ap is not None else P[name][:]))
                return t
            t_w1p = cload("w1p", [128, ncp * 128], BF16,
                          P["w1p"][:].rearrange("p c h -> p (c h)"))
            t_w1a = cload("w1a", [128, nca * 128], BF16,
                          P["w1a"][:].rearrange("p c h -> p (c h)"))
            t_b1p = cload("b1p", [128, 1], F32)
            t_b1a = cload("b1a", [128, 1], F32)
            t_wkv_c = cload("wkv_c", [128, 256], BF16)
            t_wkv_w = cload("wkv_w", [128, 256], BF16)
            t_wq = cload("wq", [128, 128], BF16)
            t_wgh = cload("wgh", [128, 2 * OUT], BF16)
            t_ident = cp.tile([128, 128], BF16)
            make_identity(nc, t_ident[:])
            t_ones = cp.tile([1, 128], BF16)
            nc.vector.memset(t_ones[:], 1.0)
            t_bkv_c = cload("bkv_c", [1, 256], BF16)
            t_bkv_w = cload("bkv_w", [1, 256], BF16)
            t_bq = cload("bq", [1, 128], BF16)
            t_bfin = cload("bfin", [1, OUT], BF16)
            t_iorow = cload("iorow", [128, 512], BF16)
            t_iocol = cload("iocol", [128, 1], F32)

            # SBUF-resident per-core tables
            qres = cp.tile([128, NWIN, 128], BF16, tag="qres")
            hpres = cp.tile([128, NWIN, 128], BF16, tag="hpres")

            # ---------------- fused pipeline over window groups ----------
            rel_c = dict(xT=P["xT_c"], dl=P["dl_c"], dlr=P["dlr_c"],
                         ctab=ctab_c, w1=t_w1p, ncx=ncp, b1=t_b1p,
                         wkv=t_wkv_c, bkv=t_bkv_c, f_bkv=f_bkv_c, tag="c")
            rel_w = dict(xT=P["xT_w"], dl=P["dl_w"], dlr=P["dlr_w"],
                         ctab=ctab_w, w1=t_w1a, ncx=nca, b1=t_b1a,
                         wkv=t_wkv_w, bkv=t_bkv_w, f_bkv=f_bkv_w, tag="w")

            with tc.tile_pool(name="sb", bufs=3) as sb, \
                 tc.tile_pool(name="p_hp", bufs=1, space="PSUM") as p_hp, \
                 tc.tile_pool(name="p_q", bufs=1, space="PSUM") as p_q, \
                 tc.tile_pool(name="p_rp", bufs=1, space="PSUM") as p_rp, \
                 tc.tile_pool(name="p_kv", bufs=1, space="PSUM") as p_kv, \
                 tc.tile_pool(name="p_ag", bufs=1, space="PSUM") as p_ag, \
                 tc.tile_pool(name="p_tp", bufs=1, space="PSUM") as p_tp, \
                 tc.tile_pool(name="p_f", bufs=1, space="PSUM") as p_f:

                def stage_a(b):
                    xt = sb.tile([128, ncp, 4, 128], BF16, tag="xt")
                    nc.sync.dma_start(
                        out=xt[:],
                        in_=P["xT_own"][b * 4:(b + 1) * 4].rearrange(
                            "t p c e -> p c t e"))
                    hp_ps = p_hp.tile([128, 4, 128], F32, tag="hp")
                    for c in range(ncp):
                        nc.tensor.matmul(
                            out=hp_ps[:].rearrange("p a b -> p (a b)"),
                            lhsT=t_w1p[:, c * 128:(c + 1) * 128],
                            rhs=xt[:, c, :, :].rearrange("p a b -> p (a b)"),
                            start=(c == 0), stop=(c == ncp - 1))
                    nc.scalar.activation(
                        out=hpres[:, b * 4:(b + 1) * 4, :].rearrange(
                            "p a b -> p (a b)"),
                        in_=hp_ps[:].rearrange("p a b -> p (a b)"),
                        func=AF.Relu, bias=t_b1p[:, :1], scale=1.0)
                    q_ps = p_q.tile([128, 4, 128], F32, tag="q")
                    for j in range(4):
                        nc.tensor.matmul(out=q_ps[:, j, :],
                                         lhsT=hpres[:, b * 4 + j, :],
                                         rhs=t_wq[:], start=True, stop=not f_bq)
                        if f_bq:
                            nc.tensor.matmul(out=q_ps[:, j, :],
                                             lhsT=t_ones[:1, :], rhs=t_bq[:1, :],
                                             start=False, stop=True)
                    nc.vector.tensor_scalar(
                        out=qres[:, b * 4:(b + 1) * 4, :].rearrange(
                            "p a b -> p (a b)"),
                        in0=q_ps[:].rearrange("p a b -> p (a b)"),
                        scalar1=0.0, scalar2=None, op0=ALU.add)

                def edge_batch(R, b):
                    w0 = b * 2
                    ncx, tag = R["ncx"], R["tag"]
                    xtb = sb.tile([128, ncx, 4, 128], BF16, tag=f"xtb_{tag}")
                    nc.scalar.dma_start(
                        out=xtb[:],
                        in_=R["xT"][b * 4:(b + 1) * 4].rearrange(
                            "t p c e -> p c t e"))
                    dl = sb.tile([128, 4], BF16, tag="dl")
                    nc.sync.dma_start(out=dl[:], in_=R["dl"][b])
                    dlr = sb.tile([1, 512], BF16, tag="dlr")
                    nc.sync.dma_start(out=dlr[:], in_=R["dlr"][b])
                    st = sb.tile([128, 4, 128], BF16, tag="st")
                    nc.vector.tensor_tensor(
                        out=st[:], in0=dl[:].to_broadcast([128, 4, 128]),
                        in1=t_iorow[:].rearrange("p (s e) -> p s e", s=4),
                        op=ALU.is_equal)
                    repl = p_rp.tile([128, 512], F32, tag="repl")
                    nc.tensor.matmul(out=repl[:], lhsT=t_ones[:1, :],
                                     rhs=dlr[:1, :], start=True, stop=True)
                    stT = sb.tile([128, 512], BF16, tag="stT")
                    nc.vector.tensor_scalar(
                        out=stT[:], in0=repl[:], scalar1=t_iocol[:, :1],
                        scalar2=None, op0=ALU.is_equal)
                    hp_ps = p_hp.tile([128, 4, 128], F32, tag="hp")
                    for c in range(ncx):
                        nc.tensor.matmul(
                            out=hp_ps[:].rearrange("p a b -> p (a b)"),
                            lhsT=R["w1"][:, c * 128:(c + 1) * 128],
                            rhs=xtb[:, c, :, :].rearrange("p a b -> p (a b)"),
                            start=(c == 0), stop=(c == ncx - 1))
                    hsT = sb.tile([128, 4, 128], BF16, tag="hsT")
                    nc.scalar.activation(
                        out=hsT[:].rearrange("p a b -> p (a b)"),
                        in_=hp_ps[:].rearrange("p a b -> p (a b)"),
                        func=AF.Relu, bias=R["b1"][:, :1], scale=1.0)
                    kv_ps = p_kv.tile([128, 4, 256], F32, tag="kv")
                    for s in range(4):
                        nc.tensor.matmul(out=kv_ps[:, s, :], lhsT=hsT[:, s, :],
                                         rhs=R["wkv"][:], start=True,
                                         stop=not R["f_bkv"])
                        if R["f_bkv"]:
                            nc.tensor.matmul(out=kv_ps[:, s, :],
                                             lhsT=t_ones[:1, :],
                                             rhs=R["bkv"][:1, :],
                                             start=False, stop=True)
                    qg_ps = p_q.tile([128, 4, 128], F32, tag="q")
                    for s in range(4):
                        nc.tensor.matmul(
                            out=qg_ps[:, s, :],
                            lhsT=stT[:, s * 128:(s + 1) * 128],
                            rhs=qres[:, w0 + s // 2, :],
                            start=True, stop=True)
                    qgs = sb.tile([128, 4, 128], BF16, tag="qgs")
                    nc.scalar.activation(
                        out=qgs[:].rearrange("p a b -> p (a b)"),
                        in_=qg_ps[:].rearrange("p a b -> p (a b)"),
                        func=AF.Copy)
                    prodb = sb.tile([128, 4, 128], BF16, tag="prodb")
                    nc.vector.tensor_tensor(
                        out=prodb[:], in0=kv_ps[:, :, 0:128], in1=qgs[:],
                        op=ALU.mult)
                    al = sb.tile([128, 16], F32, tag="al")
                    nc.vector.tensor_reduce(
                        out=al[:],
                        in_=prodb[:].rearrange("p s (h x) -> p (s h) x", x=D),
                        axis=mybir.AxisListType.X, op=ALU.add)
                    rhse = sb.tile([128, 4, 132], BF16, tag="rhse")
                    nc.scalar.activation(
                        out=rhse[:, :, 128:132],
                        in_=al[:].rearrange("p (s h) -> p s h", h=H),
                        func=AF.Exp)
                    nc.vector.tensor_tensor(
                        out=rhse[:, :, 0:128].rearrange(
                            "p s (h x) -> p s h x", x=D),
                        in0=kv_ps[:, :, 128:256].rearrange(
                            "p s (h x) -> p s h x", x=D),
                        in1=rhse[:, :, 128:132].to_broadcast([128, 4, H, D]),
                        op=ALU.mult)
                    ctrb = sb.tile([128, 2, 136], BF16, tag="ctrb")
                    for wloc in range(2):
                        agg = p_ag.tile([128, 132], F32, tag="agg")
                        for s in (2 * wloc, 2 * wloc + 1):
                            nc.tensor.matmul(
                                out=agg[:], lhsT=st[:, s, :],
                                rhs=rhse[:, s, :], start=(s % 2 == 0),
                                stop=(s % 2 == 1))
                        nc.vector.tensor_scalar(
                            out=ctrb[:, wloc, 0:132], in0=agg[:],
                            scalar1=0.0, scalar2=None, op0=ALU.add)
                    nc.sync.dma_start(
                        out=R["ctab"][w0:w0 + 2].rearrange("w p e -> p w e"),
                        in_=ctrb[:])
                    if debug and tag == "c":
                        nc.sync.dma_start(
                            out=dbg_cc[:].rearrange(
                                "(w p) e -> p w e", p=128)[:, w0:w0 + 2, :],
                            in_=ctrb[:])

                def final_batch(b):
                    g_c = sb.tile([128, 4, 136], BF16, tag="gc")
                    nc.sync.dma_start(
                        out=g_c[:],
                        in_=ctab_c[b * 4:(b + 1) * 4].rearrange("w p e -> p w e"))
                    g_w = sb.tile([128, 4, 136], BF16, tag="gw")
                    nc.sync.dma_start(
                        out=g_w[:],
                        in_=ctab_w[b * 4:(b + 1) * 4].rearrange("w p e -> p w e"))
                    rcp = sb.tile([128, 2, 16], BF16, tag="rcp")
                    for i, g in enumerate((g_c, g_w)):
                        den = sb.tile([128, 4, 4], F32, tag=f"den{i}")
                        nc.vector.tensor_scalar(
                            out=den[:], in0=g[:, :, 128:132],
                            scalar1=1e-6, scalar2=None, op0=ALU.add)
                        with nc.allow_low_precision(reason="bf16 softmax denom"):
                            nc.vector.reciprocal(
                                out=rcp[:, i, :],
                                in_=den[:].rearrange("p a b -> p (a b)"))
                    opb = sb.tile([128, 4, 128], BF16, tag="opb")
                    op2 = sb.tile([128, 4, 128], BF16, tag="op2")
                    for i, (g, o) in enumerate(((g_c, opb), (g_w, op2))):
                        nc.vector.tensor_tensor(
                            out=o[:].rearrange("p s (h x) -> p s h x", x=D),
                            in0=g[:, :, 0:128].rearrange(
                                "p s (h x) -> p s h x", x=D),
                            in1=rcp[:, i, :].rearrange("p (s h) -> p s h", h=H)
                                .to_broadcast([128, 4, H, D]),
                            op=ALU.mult)
                    nc.vector.tensor_tensor(
                        out=opb[:], in0=opb[:], in1=op2[:], op=ALU.add)
                    tp = p_tp.tile([128, 4, 128], BF16, tag="tp")
                    for j in range(4):
                        nc.tensor.transpose(out=tp[:, j, :], in_=opb[:, j, :],
                                            identity=t_ident[:])
                    gl = sb.tile([128, 4, 128], BF16, tag="gl")
                    nc.scalar.activation(
                        out=gl[:].rearrange("p a b -> p (a b)"),
                        in_=tp[:].rearrange("p a b -> p (a b)"), func=AF.Gelu)
                    f_ps = p_f.tile([128, 4, OUT], F32, tag="f")
                    for j in range(4):
                        nc.tensor.matmul(out=f_ps[:, j, :], lhsT=gl[:, j, :],
                                         rhs=t_wgh[:, 0:OUT], start=True,
                                         stop=False)
                        nc.tensor.matmul(out=f_ps[:, j, :],
                                         lhsT=hpres[:, b * 4 + j, :],
                                         rhs=t_wgh[:, OUT:2 * OUT],
                                         start=False, stop=not f_bfin)
                        if f_bfin:
                            nc.tensor.matmul(out=f_ps[:, j, :],
                                             lhsT=t_ones[:1, :],
                                             rhs=t_bfin[:1, :],
                                             start=False, stop=True)
                    fo = sb.tile([128, 4, OUT], F32, tag="fo")
                    nc.vector.tensor_scalar(
                        out=fo[:].rearrange("p a b -> p (a b)"),
                        in0=f_ps[:].rearrange("p a b -> p (a b)"),
                        scalar1=0.0, scalar2=None, op0=ALU.add)
                    nc.sync.dma_start(
                        out=out_t[b * 4 * 128:(b + 1) * 4 * 128, :].rearrange(
                            "(t p) e -> p t e", p=128),
                        in_=fo[:])

                for j in range(NWIN // 4):
                    stage_a(j)
                    for b in (2 * j, 2 * j + 1):
                        edge_batch(rel_c, b)
                        edge_batch(rel_w, b)
                    final_batch(j)

            if debug:
                nc.sync.dma_start(
                    out=dbg_q[:].rearrange("(w p) e -> p w e", p=128),
                    in_=qres[:])

    nc.compile()
    return nc


# ---------------------------------------------------------------- entry

def _prepare_all(inputs):
    fold = _fold_weights(inputs)
    xp = np.asarray(inputs["x_paper"], np.float32).astype(BF)
    xa = np.asarray(inputs["x_author"], np.float32).astype(BF)
    xp_aug = np.vstack([xp, np.zeros((1, FP), BF)])
    xa_aug = np.vstack([xa, np.zeros((1, FA), BF)])
    edges = dict(
        ws=np.asarray(inputs["writes_src"]), wd=np.asarray(inputs["writes_dst"]),
        cs=np.asarray(inputs["cites_src"]), cd=np.asarray(inputs["cites_dst"]))
    # uniform sub-tiles-per-window across cores/relations (SPMD)
    s_max = 2
    for m in range(NCORE):
        lo, hi = m * SH, (m + 1) * SH
        for dst in (edges["wd"], edges["cd"]):
            dsel = dst[(dst >= lo) & (dst < hi)] - lo
            cnt = np.bincount(dsel >> 7, minlength=NWIN).max()
            s_max = max(s_max, int(-(-cnt // 128)))
    shared = {k: fold[k] for k in (
        "w1p", "w1a", "b1p", "b1a", "wkv_c", "wkv_w", "bkv_c", "bkv_w",
        "wq", "bq", "wgh", "bfin", "iorow", "iocol")}
    in_maps = []
    for m in range(NCORE):
        core = _prep_core(m, edges, xp_aug, xa_aug, s_max)
        core.update(shared)
        in_maps.append(core)
    return fold, in_maps, s_max


def kernel(**inputs):
    fold, in_maps, s_max = _prepare_all(inputs)
    key = (fold["flags"], s_max)
    if key not in _BUILD_CACHE:
        _BUILD_CACHE[key] = build_program(s_max, fold["flags"])
    nc = _BUILD_CACHE[key]
    res = run_bass_kernel_spmd(nc, in_maps, core_ids=list(range(NCORE)))
    return np.concatenate(
        [np.asarray(res.results[m]["out"])[:SH] for m in range(NCORE)],
        axis=0)
